# revision 38
# baseline (speedup 1.0000x reference)
"""Trainium2 Bass kernel for the pairwise-distance cluster margin loss —
symmetric (triangle) decomposition.

dist2 is symmetric, so only the upper triangle of the 8x8 grid of
[512,512] blocks is computed. Each core c handles:
    slab 0: diag block (c, c)            - direct (row) stats only
    slabs 1-3: blocks (c, c+k mod 8)     - direct stats + column stats
    slab 4: half of block (c, c+4 mod 8) - two [256,256] quadrants
      (cores 0-3 take the (top-left, bottom-right) quadrants in natural
       column order; cores 4-7 get their column halves swapped by the
       host so the same program covers (top-right, bottom-left))
Per [128 x W] PSUM tile a = dist2 + C*mask (fp8 DoubleRow chain + one
bf16 aug matmul):
    aS = act_copy(a, bias=-C)  -> fp16 SBUF   (in-class: dist2, out: -inf)
    gS = (a * -1) + m2C        -> fp16 SBUF   (in-class: -dist2, out/diag: -inf)
    f_dir = rowmax(aS)  (DVE, Scalar feeds)   = far2 partial
    g_dir = rowmax(gS)  (DVE)                 = -near2 partial
with m2C = 2C*mask - C - 2^31*diag (bf16-exact). The column stats (= row
stats of the mirrored blocks, by symmetry) are folded on the host: the
packed fp16 aS/gS slab tiles are DMA'd out whole, overlapped with the
stream, and numpy maxes over the partitions. fp16 rounds dist2 to ~ulp 4
at 4096; the induced loss error is ~1e-4, well under the 2e-2 gate. The
host merges the partial maxima into per-row far2/near2, then sqrt / relu
/ mean.
"""

import numpy as np
import ml_dtypes

BF = ml_dtypes.bfloat16
F8 = ml_dtypes.float8_e4m3

N = 4096
D = 2048
P = 128
NCORES = 8
MB = N // NCORES  # 512
KX = D // P  # 16
MT = MB // P  # 4
NCLS = 64
SL = 5  # slabs per core: diag, 3 off-diag, 1 half (quadrant pair)

C = float(2.0**17)
DIAG = -float(2.0**31)


def _dcol(s, mt):
    return s * MT + mt


# Column stats (row stats of the mirrored blocks) are folded on the HOST:
# the packed fp16 aS/gS slab tiles for slabs 1-4 are DMA'd out whole
# (4.5MB/core, fully overlapped with the compute stream) and numpy takes
# the max over the 128 partitions. Every on-device alternative measured
# worse: gpsimd partition_all_reduce ~1.8us per [128,512], XBAR
# DMA-transpose ~1.2us fixed each, DVE reduces are element-bound.

_compiled = None


def _build_nc():
    import concourse.mybir as mybir
    import concourse.tile as tile
    from concourse import bacc
    from concourse.bass import ts

    nc = bacc.Bacc("TRN2", target_bir_lowering=False)
    f32 = mybir.dt.float32
    bf16 = mybir.dt.bfloat16
    fp16 = mybir.dt.float16
    fp8 = mybir.dt.float8e4
    DR = mybir.MatmulPerfMode.DoubleRow
    Copy = mybir.ActivationFunctionType.Copy
    ALU = mybir.AluOpType

    rhs8_d = nc.dram_tensor("rhs8", [SL, P, KX, 512], fp8, kind="ExternalInput")
    rhsa_d = nc.dram_tensor("rhsa", [SL, P, 512], bf16, kind="ExternalInput")
    lhs8_d = nc.dram_tensor("lhs8", [P, KX, MB], fp8, kind="ExternalInput")
    lhsaa_d = nc.dram_tensor("lhsaa", [P, MB], bf16, kind="ExternalInput")
    m2c_d = nc.dram_tensor("m2c", [SL, P, MT, 512], bf16, kind="ExternalInput")
    resd_d = nc.dram_tensor("resd", [2, P, SL * MT], f32, kind="ExternalOutput")
    aso_d = nc.dram_tensor("aso", [SL - 1, P, 2048], fp16, kind="ExternalOutput")
    gso_d = nc.dram_tensor("gso", [SL - 1, P, 2048], fp16, kind="ExternalOutput")

    X = mybir.AxisListType.X

    with tile.TileContext(nc) as tc:
        with (
            tc.tile_pool(name="singles", bufs=1) as singles,
            tc.tile_pool(name="rhsp", bufs=3) as rhsp,
            tc.tile_pool(name="rhap", bufs=2) as rhap,
            tc.tile_pool(name="m2p", bufs=2) as m2p,
            tc.tile_pool(name="psa", bufs=4, space="PSUM") as psa,
            tc.tile_pool(name="psw", bufs=1, space="PSUM") as psw,
            tc.tile_pool(name="asp", bufs=2) as asp,
            tc.tile_pool(name="gsp", bufs=2) as gsp,
        ):
            lhs8 = singles.tile([P, KX, MB], fp8)
            rhs0 = rhsp.tile([P, KX, 512], fp8, name="rhs0")
            # PE pstate warm-up: the tensor engine needs ~3us of continuous
            # work to reach full clock. Scratch matmuls on a memset tile
            # fill the otherwise-idle head (waiting on the first DMAs) so
            # the real chains start at full speed.
            warm = singles.tile([P, 512], fp8, name="warm")
            nc.gpsimd.memset(warm, 0.0)
            warmps = psw.tile([P, 512], f32, tag="warmps")
            for _ in range(11):
                nc.tensor.matmul(warmps, warm[:, 0:P], warm, start=True, stop=True)
            # first psA chain's deps first, in fine-grained pieces so the
            # DR chain never outruns the DMA; lhs pieces on gpsimd and rhs
            # pieces on sync so they land in parallel. Scalar issues no
            # DMAs - a dma_start backlog there delays the whole stat
            # pipeline behind the activations.
            nc.gpsimd.dma_start(out=lhs8[:, 0:1, :], in_=lhs8_d[:, 0:1, :])
            nc.sync.dma_start(out=rhs0[:, 0:1, :], in_=rhs8_d[0, :, 0:1, :])
            nc.gpsimd.dma_start(out=lhs8[:, 1:3, :], in_=lhs8_d[:, 1:3, :])
            nc.sync.dma_start(out=rhs0[:, 1:3, :], in_=rhs8_d[0, :, 1:3, :])
            nc.gpsimd.dma_start(out=lhs8[:, 3:6, :], in_=lhs8_d[:, 3:6, :])
            nc.sync.dma_start(out=rhs0[:, 3:6, :], in_=rhs8_d[0, :, 3:6, :])
            nc.gpsimd.dma_start(out=lhs8[:, 6:10, :], in_=lhs8_d[:, 6:10, :])
            nc.sync.dma_start(out=rhs0[:, 6:10, :], in_=rhs8_d[0, :, 6:10, :])
            nc.gpsimd.dma_start(out=lhs8[:, 10:KX, :], in_=lhs8_d[:, 10:KX, :])
            nc.sync.dma_start(out=rhs0[:, 10:KX, :], in_=rhs8_d[0, :, 10:KX, :])
            rha0 = rhap.tile([P, 512], bf16, name="rha0")
            nc.gpsimd.dma_start(out=rha0, in_=rhsa_d[0])
            lhsaa = singles.tile([P, MB], bf16)
            nc.gpsimd.dma_start(out=lhsaa, in_=lhsaa_d[:, :])
            m20 = m2p.tile([P, MT, 512], bf16, name="m20")
            nc.gpsimd.dma_start(out=m20, in_=m2c_d[0])

            fstats = singles.tile([P, SL * MT], f32, name="fstats")
            gstats = singles.tile([P, SL * MT], f32, name="gstats")

            for s in range(SL):
                if s == 0:
                    rhs, rha, m2 = rhs0, rha0, m20
                else:
                    rhs = rhsp.tile([P, KX, 512], fp8, tag="rhs0", name="rhsl")
                    # slab 1's first half on the (idle) gpsimd queue so it
                    # lands in parallel with slab 0's pieces on sync —
                    # closes a ~3us PE stall at the slab 0 -> 1 boundary
                    eng1 = nc.gpsimd if s == 1 else nc.sync
                    eng1.dma_start(out=rhs[:, 0:8, :], in_=rhs8_d[s, :, 0:8, :])
                    nc.sync.dma_start(out=rhs[:, 8:KX, :], in_=rhs8_d[s, :, 8:KX, :])
                    rha = rhap.tile([P, 512], bf16, tag="rha0", name="rhal")
                    nc.sync.dma_start(out=rha, in_=rhsa_d[s])
                    m2 = m2p.tile([P, MT, 512], bf16, tag="m20", name="m2l")
                    nc.sync.dma_start(out=m2, in_=m2c_d[s])

                # packed per-slab fp16 stat sources (column stats need the
                # whole slab in one tile for a single partition_all_reduce)
                SW = 2048 if s < 4 else 1024
                aS = asp.tile([P, 2048], fp16, tag="aS", name="aS")
                gS = gsp.tile([P, 2048], fp16, tag="gS", name="gS")

                for mt in range(MT):
                    if s < 4:
                        c0, c1 = 0, 512
                        o0 = mt * 512
                    else:
                        # quadrants: mt 0,1 -> cols [0:256); mt 2,3 -> [256:512)
                        c0, c1 = (0, 256) if mt < 2 else (256, 512)
                        o0 = mt * 256
                    W = c1 - c0
                    a = psa.tile([P, W], f32, tag="psa")
                    if s == 0 and mt == 0:
                        nc.tensor.matmul(
                            a, lhs8[:, 0, ts(mt, P)], rhs[:, 0, c0:c1],
                            start=True, stop=False,
                        )
                        for c in range(1, KX - 1, 2):
                            nc.tensor.matmul(
                                a,
                                lhs8[:, c : c + 2, ts(mt, P)],
                                rhs[:, c : c + 2, c0:c1],
                                start=False, stop=False, perf_mode=DR,
                            )
                        nc.tensor.matmul(
                            a, lhs8[:, KX - 1, ts(mt, P)], rhs[:, KX - 1, c0:c1],
                            start=False, stop=False,
                        )
                    else:
                        for c in range(0, KX, 2):
                            nc.tensor.matmul(
                                a,
                                lhs8[:, c : c + 2, ts(mt, P)],
                                rhs[:, c : c + 2, c0:c1],
                                start=(c == 0), stop=False, perf_mode=DR,
                            )
                    nc.tensor.matmul(
                        a, lhsaa[:, ts(mt, P)], rha[:, c0:c1],
                        start=False, stop=True,
                    )

                    nc.scalar.activation(aS[:, o0 : o0 + W], a, Copy, bias=-C)
                    # (tensor_tensor_reduce would fuse the sub+reduce, but
                    # that op dies at NRT execution on this compile path)
                    nc.vector.scalar_tensor_tensor(
                        gS[:, o0 : o0 + W], a, -1.0, m2[:, mt, c0:c1],
                        ALU.mult, ALU.add,
                    )
                    col = _dcol(s, mt)
                    nc.vector.reduce_max(
                        fstats[:, col : col + 1], aS[:, o0 : o0 + W], axis=X
                    )
                    nc.vector.reduce_max(
                        gstats[:, col : col + 1], gS[:, o0 : o0 + W], axis=X
                    )
                if s >= 1:
                    # ship the packed fp16 stat tiles; host does the colmax
                    nc.sync.dma_start(out=aso_d[s - 1, :, 0:SW], in_=aS[:, 0:SW])
                    nc.sync.dma_start(out=gso_d[s - 1, :, 0:SW], in_=gS[:, 0:SW])

            nc.sync.dma_start(out=resd_d[0], in_=fstats)
            nc.sync.dma_start(out=resd_d[1], in_=gstats)

    nc.compile()
    return nc


def _covers(c0):
    """For core c0, yield (s, mt, row0, gcol, w): the tile's 128 global
    rows start at row0; its columns map to global [gcol, gcol+w)."""
    out = []
    for s in range(SL):
        g = (c0 + s) % NCORES
        for mt in range(MT):
            row0 = c0 * MB + mt * P
            if s < 4:
                out.append((s, mt, row0, g * 512, 512))
            else:
                lo_half = 0 if c0 < 4 else 256
                hi_half = 256 - lo_half
                if mt < 2:
                    out.append((s, mt, row0, g * 512 + lo_half, 256))
                else:
                    out.append((s, mt, row0, g * 512 + hi_half, 256))
    return out


def _prep_inputs(x, t):
    x = np.asarray(x, np.float32)
    t = np.asarray(t).astype(np.int64)
    sq = np.sum(x.astype(np.float64) ** 2, axis=1)
    sqhi = sq.astype(BF)
    sqlo = (sq - sqhi.astype(np.float64)).astype(BF)

    ohT = np.zeros((NCLS, N), BF)
    ohT[t, np.arange(N)] = BF(1.0)

    R8 = np.ascontiguousarray((-2.0 * x).astype(F8).T).reshape(KX, P, N)
    L8 = np.ascontiguousarray(x.astype(F8).T).reshape(KX, P, N)

    RA = np.zeros((P, N), BF)
    RA[0] = sqhi
    RA[1] = sqlo
    RA[2] = BF(1.0)
    RA[3] = BF(1.0)
    RA[4 : 4 + NCLS] = (C * ohT.astype(np.float32)).astype(BF)

    LAA = np.zeros((P, N), BF)
    LAA[0] = BF(1.0)
    LAA[1] = BF(1.0)
    LAA[2] = sqhi
    LAA[3] = sqlo
    LAA[4 : 4 + NCLS] = ohT

    # m2C = 2C*mask - C - 2^31*diag  (bf16-exact: +C / -C / -2^31)
    mask_full = t[:, None] == t[None, :]
    m2c_full = np.where(mask_full, np.float32(C), np.float32(-C))
    m2c_full[np.arange(N), np.arange(N)] = np.float32(DIAG)
    m2c_full = m2c_full.astype(BF)

    in_maps = []
    for c0 in range(NCORES):
        sl = slice(c0 * MB, (c0 + 1) * MB)
        l8 = np.ascontiguousarray(L8[:, :, sl].transpose(1, 0, 2))
        laa = np.ascontiguousarray(LAA[:, sl])

        rhs8_c = np.empty((SL, P, KX, 512), F8)
        rhsa_c = np.empty((SL, P, 512), BF)
        m2c_c = np.empty((SL, P, MT, 512), BF)
        for s in range(SL):
            g = (c0 + s) % NCORES
            cols = np.arange(g * 512, (g + 1) * 512)
            if s == 4 and c0 >= 4:
                cols = np.concatenate([cols[256:], cols[:256]])
            rhs8_c[s] = R8[:, :, cols].transpose(1, 0, 2)
            rhsa_c[s] = RA[:, cols]
            m2c_c[s] = (
                m2c_full[sl][:, cols].reshape(MT, P, 512).transpose(1, 0, 2)
            )
        in_maps.append(
            {
                "rhs8": np.ascontiguousarray(rhs8_c),
                "rhsa": np.ascontiguousarray(rhsa_c),
                "lhs8": l8,
                "lhsaa": laa,
                "m2c": np.ascontiguousarray(m2c_c),
            }
        )
    return in_maps


def _assemble(results):
    far2 = np.full(N, -np.inf)
    near2n = np.full(N, -np.inf)  # holds max of (-near2) partials
    for c0 in range(NCORES):
        rd = np.asarray(results[c0]["resd"], np.float64)  # [2, P, SL*MT]
        # host-side column max over the shipped fp16 stat tiles
        aso = np.asarray(results[c0]["aso"], np.float32)  # [SL-1, P, 2048]
        gso = np.asarray(results[c0]["gso"], np.float32)
        acol = aso.max(axis=1).astype(np.float64)  # [SL-1, 2048]
        gcolm = gso.max(axis=1).astype(np.float64)
        for (s, mt, row0, gcol, w) in _covers(c0):
            col = _dcol(s, mt)
            rows = row0 + np.arange(P)
            far2[rows] = np.maximum(far2[rows], rd[0, :, col])
            near2n[rows] = np.maximum(near2n[rows], rd[1, :, col])
            if s >= 1:
                o0 = mt * (512 if s < 4 else 256)
                crows = gcol + np.arange(w)
                far2[crows] = np.maximum(far2[crows], acol[s - 1, o0 : o0 + w])
                near2n[crows] = np.maximum(
                    near2n[crows], gcolm[s - 1, o0 : o0 + w]
                )
    near2 = -near2n
    far = np.sqrt(np.maximum(far2, 0.0))
    near = np.sqrt(np.maximum(near2, 0.0))
    loss = np.float32(np.mean(np.maximum(far - near, 0.0)))
    return np.asarray(loss, np.float32)


def run_kernel(inputs, targets, trace=False):
    from concourse.bass_utils import run_bass_kernel_spmd

    global _compiled
    if _compiled is None:
        _compiled = _build_nc()
    nc = _compiled
    in_maps = _prep_inputs(inputs, targets)
    br = run_bass_kernel_spmd(
        nc, in_maps, core_ids=list(range(NCORES)), trace=trace
    )
    return _assemble(br.results), br


def kernel(inputs, targets):
    loss, _ = run_kernel(inputs, targets)
    return loss


# revision 40
# speedup vs baseline: 1.1432x; 1.1432x over previous
"""Trainium2 Bass kernel for the pairwise-distance cluster margin loss —
symmetric (triangle) decomposition.

dist2 is symmetric, so only the upper triangle of the 8x8 grid of
[512,512] blocks is computed. Each core c handles:
    slab 0: diag block (c, c)            - direct (row) stats only
    slabs 1-3: blocks (c, c+k mod 8)     - direct stats + column stats
    slab 4: half of block (c, c+4 mod 8) - two [256,256] quadrants
      (cores 0-3 take the (top-left, bottom-right) quadrants in natural
       column order; cores 4-7 get their column halves swapped by the
       host so the same program covers (top-right, bottom-left))
Per [128 x W] PSUM tile a = dist2 + C*mask (fp8 DoubleRow chain + one
bf16 aug matmul):
    aS = act_copy(a, bias=-C)  -> fp16 SBUF   (in-class: dist2, out: -inf)
    gS = (a * -1) + m2C        -> fp16 SBUF   (in-class: -dist2, out/diag: -inf)
    f_dir = rowmax(aS)  (DVE, Scalar feeds)   = far2 partial
    g_dir = rowmax(gS)  (DVE)                 = -near2 partial
with m2C = 2C*mask - C - 2^31*diag (bf16-exact). The column stats (= row
stats of the mirrored blocks, by symmetry) are folded on the host: the
packed fp16 aS/gS slab tiles are DMA'd out whole, overlapped with the
stream, and numpy maxes over the partitions. fp16 rounds dist2 to ~ulp 4
at 4096; the induced loss error is ~1e-4, well under the 2e-2 gate. The
host merges the partial maxima into per-row far2/near2, then sqrt / relu
/ mean.
"""

import numpy as np
import ml_dtypes

BF = ml_dtypes.bfloat16
F8 = ml_dtypes.float8_e4m3

N = 4096
D = 2048
P = 128
NCORES = 8
MB = N // NCORES  # 512
KX = D // P  # 16
MT = MB // P  # 4
NCLS = 64
SL = 5  # slabs per core: diag, 3 off-diag, 1 half (quadrant pair)

C = float(2.0**17)
DIAG = -float(2.0**31)


def _dcol(s, mt):
    return s * MT + mt


# Column stats (row stats of the mirrored blocks) are folded on the HOST:
# the packed fp16 aS/gS slab tiles for slabs 1-4 are DMA'd out whole
# (4.5MB/core, fully overlapped with the compute stream) and numpy takes
# the max over the 128 partitions. Every on-device alternative measured
# worse: gpsimd partition_all_reduce ~1.8us per [128,512], XBAR
# DMA-transpose ~1.2us fixed each, DVE reduces are element-bound.

_compiled = None


def _build_nc():
    import concourse.mybir as mybir
    import concourse.tile as tile
    from concourse import bacc
    from concourse.bass import ts

    nc = bacc.Bacc("TRN2", target_bir_lowering=False)
    f32 = mybir.dt.float32
    bf16 = mybir.dt.bfloat16
    fp16 = mybir.dt.float16
    fp8 = mybir.dt.float8e4
    DR = mybir.MatmulPerfMode.DoubleRow
    Copy = mybir.ActivationFunctionType.Copy
    ALU = mybir.AluOpType

    rhs8_d = nc.dram_tensor("rhs8", [SL, P, KX, 512], fp8, kind="ExternalInput")
    rhsa_d = nc.dram_tensor("rhsa", [SL, P, 512], bf16, kind="ExternalInput")
    lhs8_d = nc.dram_tensor("lhs8", [P, KX, MB], fp8, kind="ExternalInput")
    lhsaa_d = nc.dram_tensor("lhsaa", [P, MB], bf16, kind="ExternalInput")
    m2c_d = nc.dram_tensor("m2c", [SL, P, MT, 512], bf16, kind="ExternalInput")
    resd_d = nc.dram_tensor("resd", [2, P, SL * MT], f32, kind="ExternalOutput")
    aso_d = nc.dram_tensor("aso", [SL - 1, P, 2048], fp16, kind="ExternalOutput")
    gso_d = nc.dram_tensor("gso", [SL - 1, P, 2048], fp16, kind="ExternalOutput")

    X = mybir.AxisListType.X

    with tile.TileContext(nc) as tc:
        with (
            tc.tile_pool(name="singles", bufs=1) as singles,
            tc.tile_pool(name="rhsp", bufs=3) as rhsp,
            tc.tile_pool(name="rhap", bufs=2) as rhap,
            tc.tile_pool(name="m2p", bufs=2) as m2p,
            tc.tile_pool(name="psa", bufs=4, space="PSUM") as psa,
            tc.tile_pool(name="psw", bufs=1, space="PSUM") as psw,
            tc.tile_pool(name="asp", bufs=2) as asp,
            tc.tile_pool(name="gsp", bufs=2) as gsp,
        ):
            lhs8 = singles.tile([P, KX, MB], fp8)
            rhs0 = rhsp.tile([P, KX, 512], fp8, name="rhs0")
            # PE pstate warm-up: the tensor engine needs ~3us of continuous
            # work to reach full clock. Scratch matmuls on a memset tile
            # fill the otherwise-idle head (waiting on the first DMAs) so
            # the real chains start at full speed.
            warm = singles.tile([P, 512], fp8, name="warm")
            nc.gpsimd.memset(warm, 0.0)
            warmps = psw.tile([P, 512], f32, tag="warmps")
            for _ in range(14):
                nc.tensor.matmul(warmps, warm[:, 0:P], warm, start=True, stop=True)
            # first psA chain's deps first, in fine-grained pieces so the
            # DR chain never outruns the DMA; lhs pieces on gpsimd and rhs
            # pieces on sync so they land in parallel. Scalar issues no
            # DMAs - a dma_start backlog there delays the whole stat
            # pipeline behind the activations.
            nc.gpsimd.dma_start(out=lhs8[:, 0:1, :], in_=lhs8_d[:, 0:1, :])
            nc.sync.dma_start(out=rhs0[:, 0:1, :], in_=rhs8_d[0, :, 0:1, :])
            nc.gpsimd.dma_start(out=lhs8[:, 1:3, :], in_=lhs8_d[:, 1:3, :])
            nc.sync.dma_start(out=rhs0[:, 1:3, :], in_=rhs8_d[0, :, 1:3, :])
            nc.gpsimd.dma_start(out=lhs8[:, 3:6, :], in_=lhs8_d[:, 3:6, :])
            nc.sync.dma_start(out=rhs0[:, 3:6, :], in_=rhs8_d[0, :, 3:6, :])
            nc.gpsimd.dma_start(out=lhs8[:, 6:10, :], in_=lhs8_d[:, 6:10, :])
            nc.sync.dma_start(out=rhs0[:, 6:10, :], in_=rhs8_d[0, :, 6:10, :])
            nc.gpsimd.dma_start(out=lhs8[:, 10:KX, :], in_=lhs8_d[:, 10:KX, :])
            nc.sync.dma_start(out=rhs0[:, 10:KX, :], in_=rhs8_d[0, :, 10:KX, :])
            rha0 = rhap.tile([P, 512], bf16, name="rha0")
            nc.gpsimd.dma_start(out=rha0, in_=rhsa_d[0])
            lhsaa = singles.tile([P, MB], bf16)
            nc.gpsimd.dma_start(out=lhsaa, in_=lhsaa_d[:, :])
            m20 = m2p.tile([P, MT, 512], bf16, name="m20")
            nc.gpsimd.dma_start(out=m20, in_=m2c_d[0])

            fstats = singles.tile([P, SL * MT], f32, name="fstats")
            gstats = singles.tile([P, SL * MT], f32, name="gstats")

            for s in range(SL):
                if s == 0:
                    rhs, rha, m2 = rhs0, rha0, m20
                else:
                    rhs = rhsp.tile([P, KX, 512], fp8, tag="rhs0", name="rhsl")
                    nc.sync.dma_start(out=rhs[:, 0:8, :], in_=rhs8_d[s, :, 0:8, :])
                    nc.sync.dma_start(out=rhs[:, 8:KX, :], in_=rhs8_d[s, :, 8:KX, :])
                    rha = rhap.tile([P, 512], bf16, tag="rha0", name="rhal")
                    nc.sync.dma_start(out=rha, in_=rhsa_d[s])
                    m2 = m2p.tile([P, MT, 512], bf16, tag="m20", name="m2l")
                    nc.sync.dma_start(out=m2, in_=m2c_d[s])

                # packed per-slab fp16 stat sources (column stats need the
                # whole slab in one tile for a single partition_all_reduce)
                SW = 2048 if s < 4 else 1024
                aS = asp.tile([P, 2048], fp16, tag="aS", name="aS")
                gS = gsp.tile([P, 2048], fp16, tag="gS", name="gS")

                for mt in range(MT):
                    if s < 4:
                        c0, c1 = 0, 512
                        o0 = mt * 512
                    else:
                        # quadrants: mt 0,1 -> cols [0:256); mt 2,3 -> [256:512)
                        c0, c1 = (0, 256) if mt < 2 else (256, 512)
                        o0 = mt * 256
                    W = c1 - c0
                    a = psa.tile([P, W], f32, tag="psa")
                    if s == 0 and mt == 0:
                        nc.tensor.matmul(
                            a, lhs8[:, 0, ts(mt, P)], rhs[:, 0, c0:c1],
                            start=True, stop=False,
                        )
                        for c in range(1, KX - 1, 2):
                            nc.tensor.matmul(
                                a,
                                lhs8[:, c : c + 2, ts(mt, P)],
                                rhs[:, c : c + 2, c0:c1],
                                start=False, stop=False, perf_mode=DR,
                            )
                        nc.tensor.matmul(
                            a, lhs8[:, KX - 1, ts(mt, P)], rhs[:, KX - 1, c0:c1],
                            start=False, stop=False,
                        )
                    else:
                        for c in range(0, KX, 2):
                            nc.tensor.matmul(
                                a,
                                lhs8[:, c : c + 2, ts(mt, P)],
                                rhs[:, c : c + 2, c0:c1],
                                start=(c == 0), stop=False, perf_mode=DR,
                            )
                    nc.tensor.matmul(
                        a, lhsaa[:, ts(mt, P)], rha[:, c0:c1],
                        start=False, stop=True,
                    )

                    nc.scalar.activation(aS[:, o0 : o0 + W], a, Copy, bias=-C)
                    # (tensor_tensor_reduce would fuse the sub+reduce, but
                    # that op dies at NRT execution on this compile path)
                    nc.vector.scalar_tensor_tensor(
                        gS[:, o0 : o0 + W], a, -1.0, m2[:, mt, c0:c1],
                        ALU.mult, ALU.add,
                    )
                    col = _dcol(s, mt)
                    nc.vector.reduce_max(
                        fstats[:, col : col + 1], aS[:, o0 : o0 + W], axis=X
                    )
                    nc.vector.reduce_max(
                        gstats[:, col : col + 1], gS[:, o0 : o0 + W], axis=X
                    )
                if s >= 1:
                    # ship the packed fp16 stat tiles; host does the colmax
                    nc.sync.dma_start(out=aso_d[s - 1, :, 0:SW], in_=aS[:, 0:SW])
                    nc.sync.dma_start(out=gso_d[s - 1, :, 0:SW], in_=gS[:, 0:SW])

            nc.sync.dma_start(out=resd_d[0], in_=fstats)
            nc.sync.dma_start(out=resd_d[1], in_=gstats)

    nc.compile()
    return nc


def _covers(c0):
    """For core c0, yield (s, mt, row0, gcol, w): the tile's 128 global
    rows start at row0; its columns map to global [gcol, gcol+w)."""
    out = []
    for s in range(SL):
        g = (c0 + s) % NCORES
        for mt in range(MT):
            row0 = c0 * MB + mt * P
            if s < 4:
                out.append((s, mt, row0, g * 512, 512))
            else:
                lo_half = 0 if c0 < 4 else 256
                hi_half = 256 - lo_half
                if mt < 2:
                    out.append((s, mt, row0, g * 512 + lo_half, 256))
                else:
                    out.append((s, mt, row0, g * 512 + hi_half, 256))
    return out


def _prep_inputs(x, t):
    x = np.asarray(x, np.float32)
    t = np.asarray(t).astype(np.int64)
    sq = np.sum(x.astype(np.float64) ** 2, axis=1)
    sqhi = sq.astype(BF)
    sqlo = (sq - sqhi.astype(np.float64)).astype(BF)

    ohT = np.zeros((NCLS, N), BF)
    ohT[t, np.arange(N)] = BF(1.0)

    R8 = np.ascontiguousarray((-2.0 * x).astype(F8).T).reshape(KX, P, N)
    L8 = np.ascontiguousarray(x.astype(F8).T).reshape(KX, P, N)

    RA = np.zeros((P, N), BF)
    RA[0] = sqhi
    RA[1] = sqlo
    RA[2] = BF(1.0)
    RA[3] = BF(1.0)
    RA[4 : 4 + NCLS] = (C * ohT.astype(np.float32)).astype(BF)

    LAA = np.zeros((P, N), BF)
    LAA[0] = BF(1.0)
    LAA[1] = BF(1.0)
    LAA[2] = sqhi
    LAA[3] = sqlo
    LAA[4 : 4 + NCLS] = ohT

    # m2C = 2C*mask - C - 2^31*diag  (bf16-exact: +C / -C / -2^31)
    mask_full = t[:, None] == t[None, :]
    m2c_full = np.where(mask_full, np.float32(C), np.float32(-C))
    m2c_full[np.arange(N), np.arange(N)] = np.float32(DIAG)
    m2c_full = m2c_full.astype(BF)

    in_maps = []
    for c0 in range(NCORES):
        sl = slice(c0 * MB, (c0 + 1) * MB)
        l8 = np.ascontiguousarray(L8[:, :, sl].transpose(1, 0, 2))
        laa = np.ascontiguousarray(LAA[:, sl])

        rhs8_c = np.empty((SL, P, KX, 512), F8)
        rhsa_c = np.empty((SL, P, 512), BF)
        m2c_c = np.empty((SL, P, MT, 512), BF)
        for s in range(SL):
            g = (c0 + s) % NCORES
            cols = np.arange(g * 512, (g + 1) * 512)
            if s == 4 and c0 >= 4:
                cols = np.concatenate([cols[256:], cols[:256]])
            rhs8_c[s] = R8[:, :, cols].transpose(1, 0, 2)
            rhsa_c[s] = RA[:, cols]
            m2c_c[s] = (
                m2c_full[sl][:, cols].reshape(MT, P, 512).transpose(1, 0, 2)
            )
        in_maps.append(
            {
                "rhs8": np.ascontiguousarray(rhs8_c),
                "rhsa": np.ascontiguousarray(rhsa_c),
                "lhs8": l8,
                "lhsaa": laa,
                "m2c": np.ascontiguousarray(m2c_c),
            }
        )
    return in_maps


def _assemble(results):
    far2 = np.full(N, -np.inf)
    near2n = np.full(N, -np.inf)  # holds max of (-near2) partials
    for c0 in range(NCORES):
        rd = np.asarray(results[c0]["resd"], np.float64)  # [2, P, SL*MT]
        # host-side column max over the shipped fp16 stat tiles
        aso = np.asarray(results[c0]["aso"], np.float32)  # [SL-1, P, 2048]
        gso = np.asarray(results[c0]["gso"], np.float32)
        acol = aso.max(axis=1).astype(np.float64)  # [SL-1, 2048]
        gcolm = gso.max(axis=1).astype(np.float64)
        for (s, mt, row0, gcol, w) in _covers(c0):
            col = _dcol(s, mt)
            rows = row0 + np.arange(P)
            far2[rows] = np.maximum(far2[rows], rd[0, :, col])
            near2n[rows] = np.maximum(near2n[rows], rd[1, :, col])
            if s >= 1:
                o0 = mt * (512 if s < 4 else 256)
                crows = gcol + np.arange(w)
                far2[crows] = np.maximum(far2[crows], acol[s - 1, o0 : o0 + w])
                near2n[crows] = np.maximum(
                    near2n[crows], gcolm[s - 1, o0 : o0 + w]
                )
    near2 = -near2n
    far = np.sqrt(np.maximum(far2, 0.0))
    near = np.sqrt(np.maximum(near2, 0.0))
    loss = np.float32(np.mean(np.maximum(far - near, 0.0)))
    return np.asarray(loss, np.float32)


def run_kernel(inputs, targets, trace=False):
    from concourse.bass_utils import run_bass_kernel_spmd

    global _compiled
    if _compiled is None:
        _compiled = _build_nc()
    nc = _compiled
    in_maps = _prep_inputs(inputs, targets)
    br = run_bass_kernel_spmd(
        nc, in_maps, core_ids=list(range(NCORES)), trace=trace
    )
    return _assemble(br.results), br


def kernel(inputs, targets):
    loss, _ = run_kernel(inputs, targets)
    return loss
